# revision 5
# baseline (speedup 1.0000x reference)
"""DualAttention Trainium2 kernel (nn_DualAttention_44341242364496).

Reference math (per batch element, X = points[b], shape (N=4096, C=256)):
  q = X Wq^T + bq ; k = X Wk^T + bk          (N, 32)
  P = softmax(q k^T, axis=-1)                (N, N)
  v = X Wv^T + bv                            (N, 256)
  out_p = gamma * P v + X
  E = X X^T contracted over rows -> (C, C):  E = X^T X
  A = softmax(max_d(E) - E, axis=-1) == stable softmax(-E)
  out_c = gamma * A X^T' (einsum bcd,bnd->bnc) + X
  out = out_p + out_c = gamma*(Pv) + gamma*(X A^T) + 2X

Distribution: 8 cores; core c handles batch b=c//2, query-row half h=c%2.
Each core receives its batch's X in two layouts (scaled by 2, with the
core's own rows ordered first), computes k/v/E for the full batch element
(duplicated with its pair core), and produces its 2048 output rows.

Softmax tricks used on-chip:
 - point branch: exp() without max subtraction (scores are O(+-40), safe in
   fp32), denominator obtained by appending a ones-column to v so the PV
   matmul also yields row sums; division folded into the output scaling.
 - channel branch: softmax(max-E) == exp(-(E - min_row)) / sum.
All matmuls run in float32r (TF32-like PE fast path, 1 cycle/row).
"""

import sys

sys.path.insert(0, "/opt/trn_rl_repo")

import numpy as np

import concourse.bass as bass  # noqa: F401  (bass types used via bacc/tile)
import concourse.mybir as mybir
import concourse.tile as tile
from concourse import bacc
from concourse.bass_utils import run_bass_kernel_spmd
from concourse.masks import make_identity

B, N, C = 4, 4096, 256
C8 = C // 8  # 32
NCORES = 8
HALF = N // 2  # 2048 query rows per core
NBLK = HALF // 128  # 16 output row blocks per core
KCH = N // 128  # 32 key chunks
P = 128

F32 = mybir.dt.float32
F32R = mybir.dt.float32r
AX = mybir.AxisListType
ALU = mybir.AluOpType
ACTF = mybir.ActivationFunctionType

_CACHE: dict = {}


def _build_nc():
    nc = bacc.Bacc("TRN2", target_bir_lowering=False)

    x2T_d = nc.dram_tensor("x2T", [P, 2, N], F32R, kind="ExternalInput")
    x2_d = nc.dram_tensor("x2", [P, KCH, C], F32R, kind="ExternalInput")
    wqT_d = nc.dram_tensor("wqT", [P, 2, C8], F32R, kind="ExternalInput")
    wkT_d = nc.dram_tensor("wkT", [P, 2, C8], F32R, kind="ExternalInput")
    wvT_d = nc.dram_tensor("wvT", [P, 2, C + 2], F32R, kind="ExternalInput")
    bq_d = nc.dram_tensor("bqr", [1, C8], F32R, kind="ExternalInput")
    bk_d = nc.dram_tensor("bkr", [1, C8], F32R, kind="ExternalInput")
    bv_d = nc.dram_tensor("bvr", [1, C + 2], F32R, kind="ExternalInput")
    gam_d = nc.dram_tensor("gam", [1, 1], F32, kind="ExternalInput")
    ones_d = nc.dram_tensor("ones", [1, 512], F32R, kind="ExternalInput")
    out_d = nc.dram_tensor("out_rows", [NBLK, P, C], F32, kind="ExternalOutput")

    with tile.TileContext(nc) as tc:
        with (
            tc.tile_pool(name="singles", bufs=1) as singles,
            tc.tile_pool(name="persist", bufs=1) as persist,
            tc.tile_pool(name="pTp", bufs=3) as pTp,
            tc.tile_pool(name="sbout", bufs=3) as sbout,
            tc.tile_pool(name="small", bufs=10) as small,
            tc.tile_pool(name="psS", bufs=2, space="PSUM") as psS,
            tc.tile_pool(name="psO", bufs=4, space="PSUM") as psO,
            tc.tile_pool(name="psC", bufs=2, space="PSUM") as psC,
        ):
            # ---------------- Phase A: loads & constants ----------------
            x2T = persist.tile([P, 2, N], F32R, tag="x2T")
            nc.sync.dma_start(x2T[:], x2T_d.ap())
            x2 = persist.tile([P, KCH, C], F32R, tag="x2")
            nc.sync.dma_start(x2[:], x2_d.ap())
            wqT = singles.tile([P, 2, C8], F32R, tag="wqT")
            nc.sync.dma_start(wqT[:], wqT_d.ap())
            wkT = singles.tile([P, 2, C8], F32R, tag="wkT")
            nc.sync.dma_start(wkT[:], wkT_d.ap())
            wvT = singles.tile([P, 2, C + 2], F32R, tag="wvT")
            nc.sync.dma_start(wvT[:], wvT_d.ap())
            bqr = singles.tile([1, C8], F32R, tag="bqr")
            nc.sync.dma_start(bqr[:], bq_d.ap())
            bkr = singles.tile([1, C8], F32R, tag="bkr")
            nc.sync.dma_start(bkr[:], bk_d.ap())
            bvr = singles.tile([1, C + 2], F32R, tag="bvr")
            nc.sync.dma_start(bvr[:], bv_d.ap())
            ones = singles.tile([1, 512], F32R, tag="ones")
            nc.sync.dma_start(ones[:], ones_d.ap())
            ident = singles.tile([P, P], F32, tag="ident")
            make_identity(nc, ident[:])
            gb = singles.tile([P, 1], F32, tag="gb")
            nc.sync.dma_start(gb[:], gam_d.ap().to_broadcast([P, 1]))
            gh = singles.tile([P, 1], F32, tag="gh")
            nc.vector.tensor_scalar_mul(gh[:], gb[:], 0.5)

            # ------- Phase B: channel attention (E = X^T X, softmax) -------
            # E' = x2^T x2 = 4E ; A = exp(-(E - min)) / sum = exp(-.25 E' + .25 min')
            attn_n = singles.tile([P, 2, C], F32, tag="attn_n")
            attnTg = persist.tile([P, 2, C], F32R, tag="attnTg")
            for cb in range(2):
                e_ps = psO.tile([P, 512], F32, tag="o", name=f"e_{cb}")[:, :C]
                for nk in range(KCH):
                    nc.tensor.matmul(
                        e_ps,
                        x2[:, nk, cb * P : (cb + 1) * P],
                        x2[:, nk, :],
                        start=(nk == 0),
                        stop=(nk == KCH - 1),
                    )
                emin = small.tile([P, 1], F32, tag="sm", name=f"emin{cb}")
                nc.vector.tensor_reduce(
                    emin[:], e_ps, axis=AX.X, op=ALU.min
                )
                emq = small.tile([P, 1], F32, tag="sm", name=f"emq{cb}")
                nc.vector.tensor_scalar_mul(emq[:], emin[:], 0.25)
                us = small.tile([P, 1], F32, tag="sm", name=f"us{cb}")
                nc.scalar.activation(
                    attn_n[:, cb, :],
                    e_ps,
                    ACTF.Exp,
                    bias=emq[:],
                    scale=-0.25,
                    accum_out=us[:],
                )
                rc = small.tile([P, 1], F32, tag="sm", name=f"rc{cb}")
                nc.vector.reciprocal(rc[:], us[:])
                rcg = small.tile([P, 1], F32, tag="sm", name=f"rcg{cb}")
                nc.vector.tensor_mul(rcg[:], rc[:], gh[:])
                nc.vector.tensor_scalar_mul(
                    attn_n[:, cb, :], attn_n[:, cb, :], rcg[:]
                )
            for dd in range(2):
                for cc in range(2):
                    t_ps = psO.tile([P, 512], F32, tag="o", name=f"t_{dd}{cc}")[
                        :, :P
                    ]
                    nc.tensor.transpose(
                        t_ps, attn_n[:, cc, dd * P : (dd + 1) * P], ident[:]
                    )
                    nc.vector.tensor_copy(
                        attnTg[:, dd, cc * P : (cc + 1) * P], t_ps
                    )

            # ---------------- Phase C: kT, qT, V ----------------
            kT = persist.tile([C8, N], F32R, tag="kT")
            qT = persist.tile([C8, HALF], F32R, tag="qT")
            for seg in range(N // 512):
                kps = psS.tile([P, 512], F32, tag="s", name=f"k_{seg}")[:C8, :]
                for cc in range(2):
                    nc.tensor.matmul(
                        kps,
                        wkT[:, cc, :],
                        x2T[:, cc, seg * 512 : (seg + 1) * 512],
                        start=(cc == 0),
                        stop=False,
                    )
                nc.tensor.matmul(kps, bkr[:], ones[:], start=False, stop=True)
                nc.vector.tensor_copy(kT[:, seg * 512 : (seg + 1) * 512], kps)
            for seg in range(HALF // 512):
                qps = psS.tile([P, 512], F32, tag="s", name=f"q_{seg}")[:C8, :]
                for cc in range(2):
                    nc.tensor.matmul(
                        qps,
                        wqT[:, cc, :],
                        x2T[:, cc, seg * 512 : (seg + 1) * 512],
                        start=(cc == 0),
                        stop=False,
                    )
                nc.tensor.matmul(qps, bqr[:], ones[:], start=False, stop=True)
                nc.vector.tensor_copy(qT[:, seg * 512 : (seg + 1) * 512], qps)
            vaug = persist.tile([P, KCH, C + 2], F32R, tag="vaug")
            for nk in range(KCH):
                vps = psS.tile([P, 512], F32, tag="s", name=f"v_{nk}")[:, : C + 2]
                for cc in range(2):
                    nc.tensor.matmul(
                        vps,
                        x2T[:, cc, nk * P : (nk + 1) * P],
                        wvT[:, cc, :],
                        start=(cc == 0),
                        stop=False,
                    )
                nc.tensor.matmul(
                    vps, ones[:, :P], bvr[:], start=False, stop=True
                )
                nc.vector.tensor_copy(vaug[:, nk, :], vps)

            # ---------------- Phase D: point attention ----------------
            for m in range(HALF // 512):
                o_ps = [
                    psO.tile([P, 512], F32, tag="o", name=f"o_{m}_{j}")[:, : C + 2]
                    for j in range(4)
                ]
                for kk in range(KCH):
                    s_ps = psS.tile([P, 512], F32, tag="s", name=f"s_{m}_{kk}")
                    nc.tensor.matmul(
                        s_ps[:],
                        kT[:, kk * P : (kk + 1) * P],
                        qT[:, m * 512 : (m + 1) * 512],
                        start=True,
                        stop=True,
                    )
                    pT = pTp.tile([P, 512], F32R, tag="pT", name=f"p_{m}_{kk}")
                    nc.scalar.activation(pT[:], s_ps[:], ACTF.Exp)
                    for j in range(4):
                        nc.tensor.matmul(
                            o_ps[j],
                            pT[:, j * P : (j + 1) * P],
                            vaug[:, kk, :],
                            start=(kk == 0),
                            stop=(kk == KCH - 1),
                        )
                for j in range(4):
                    blk = m * 4 + j
                    c_ps = psC.tile([P, 512], F32, tag="c", name=f"c_{blk}")[
                        :, :C
                    ]
                    for dd in range(2):
                        nc.tensor.matmul(
                            c_ps,
                            x2T[:, dd, blk * P : (blk + 1) * P],
                            attnTg[:, dd, :],
                            start=(dd == 0),
                            stop=(dd == 1),
                        )
                    sq = small.tile([P, 1], F32, tag="sm", name=f"sq{blk}")
                    nc.vector.tensor_copy(sq[:], o_ps[j][:, C : C + 1])
                    rq = small.tile([P, 1], F32, tag="sm", name=f"rq{blk}")
                    nc.vector.reciprocal(rq[:], sq[:])
                    rqg = small.tile([P, 1], F32, tag="sm", name=f"rqg{blk}")
                    nc.vector.tensor_mul(rqg[:], rq[:], gb[:])
                    acc = sbout.tile([P, C], F32, tag="acc", name=f"acc{blk}")
                    nc.vector.tensor_scalar_mul(acc[:], o_ps[j][:, :C], rqg[:])
                    nc.vector.tensor_add(acc[:], acc[:], c_ps)
                    nc.vector.tensor_add(
                        acc[:], acc[:], x2[:, blk, :].bitcast(F32)
                    )
                    nc.sync.dma_start(out_d.ap()[blk], acc[:])

    nc.compile()
    return nc


def _prep_core_inputs(points, Wq, bq, Wk, bk, Wv, bv, gamma, core):
    b, h = core // 2, core % 2
    xb = np.asarray(points[b], dtype=np.float32)
    # own rows first, then the other half (key order is softmax-invariant
    # as long as kT and v use the same order, which they do)
    xp = np.concatenate([xb[h * HALF : (h + 1) * HALF], xb[(1 - h) * HALF : (2 - h) * HALF]])
    x2 = (2.0 * xp).reshape(KCH, P, C).transpose(1, 0, 2).copy()  # (128, 32, 256)
    x2T = np.ascontiguousarray(
        (2.0 * xp).T.reshape(2, P, N).transpose(1, 0, 2)
    )  # (128, 2, 4096)
    return {"x2T": x2T, "x2": x2}


def _prep_shared_inputs(Wq, bq, Wk, bk, Wv, bv, gamma):
    wqT = np.ascontiguousarray((0.5 * np.asarray(Wq, np.float32).T).reshape(2, P, C8).transpose(1, 0, 2))
    wkT = np.ascontiguousarray((0.5 * np.asarray(Wk, np.float32).T).reshape(2, P, C8).transpose(1, 0, 2))
    wvT_full = 0.5 * np.asarray(Wv, np.float32).T  # (256, 256)
    wvT_aug = np.concatenate(
        [wvT_full, np.zeros((C, 2), np.float32)], axis=1
    )  # (256, 258)
    wvT = np.ascontiguousarray(wvT_aug.reshape(2, P, C + 2).transpose(1, 0, 2))
    bv_aug = np.concatenate([np.asarray(bv, np.float32), [1.0, 0.0]]).reshape(1, C + 2)
    return {
        "wqT": wqT,
        "wkT": wkT,
        "wvT": wvT,
        "bqr": np.asarray(bq, np.float32).reshape(1, C8),
        "bkr": np.asarray(bk, np.float32).reshape(1, C8),
        "bvr": bv_aug,
        "gam": np.asarray(gamma, np.float32).reshape(1, 1),
        "ones": np.ones((1, 512), np.float32),
    }


def kernel(points, Wq, bq, Wk, bk, Wv, bv, gamma, **run_kwargs):
    if "nc" not in _CACHE:
        _CACHE["nc"] = _build_nc()
    nc = _CACHE["nc"]

    shared = _prep_shared_inputs(Wq, bq, Wk, bk, Wv, bv, gamma)
    in_maps = []
    for core in range(NCORES):
        m = dict(shared)
        m.update(_prep_core_inputs(points, Wq, bq, Wk, bk, Wv, bv, gamma, core))
        in_maps.append(m)

    res = run_bass_kernel_spmd(
        nc, in_maps, core_ids=list(range(NCORES)), **run_kwargs
    )
    out = np.empty((B, N, C), dtype=np.float32)
    for core in range(NCORES):
        b, h = core // 2, core % 2
        out[b, h * HALF : (h + 1) * HALF] = (
            res.results[core]["out_rows"].reshape(HALF, C)
        )
    if run_kwargs:
        kernel.last_results = res  # expose profile info to test harness
    return out


# revision 6
# speedup vs baseline: 1.2150x; 1.2150x over previous
"""DualAttention Trainium2 kernel (nn_DualAttention_44341242364496).

Reference math (per batch element, X = points[b], shape (N=4096, C=256)):
  q = X Wq^T + bq ; k = X Wk^T + bk          (N, 32)
  P = softmax(q k^T, axis=-1)                (N, N)
  v = X Wv^T + bv                            (N, 256)
  out_p = gamma * P v + X
  E = X X^T contracted over rows -> (C, C):  E = X^T X
  A = softmax(max_d(E) - E, axis=-1) == stable softmax(-E)
  out_c = gamma * A X^T' (einsum bcd,bnd->bnc) + X
  out = out_p + out_c = gamma*(Pv) + gamma*(X A^T) + 2X

Distribution: 8 cores; core c handles batch b=c//2, query-row half h=c%2.
Each core receives its batch's X in two layouts (scaled by 2, with the
core's own rows ordered first), computes k/v/E for the full batch element
(duplicated with its pair core), and produces its 2048 output rows.

Softmax tricks used on-chip:
 - point branch: exp() without max subtraction (scores are O(+-40), safe in
   fp32), denominator obtained by appending a ones-column to v so the PV
   matmul also yields row sums; division folded into the output scaling.
 - channel branch: softmax(max-E) == exp(-(E - min_row)) / sum.
All matmuls run in float32r (TF32-like PE fast path, 1 cycle/row).
"""

import sys

sys.path.insert(0, "/opt/trn_rl_repo")

import numpy as np
import ml_dtypes

import concourse.bass as bass  # noqa: F401  (bass types used via bacc/tile)
import concourse.mybir as mybir
import concourse.tile as tile
from concourse import bacc
from concourse.bass_utils import run_bass_kernel_spmd
from concourse.masks import make_identity

B, N, C = 4, 4096, 256
C8 = C // 8  # 32
NCORES = 8
HALF = N // 2  # 2048 query rows per core
NBLK = HALF // 128  # 16 output row blocks per core
KCH = N // 128  # 32 key chunks
P = 128

F32 = mybir.dt.float32
F32R = mybir.dt.float32r
BF16 = mybir.dt.bfloat16
AX = mybir.AxisListType
ALU = mybir.AluOpType
ACTF = mybir.ActivationFunctionType

_CACHE: dict = {}


def _build_nc():
    nc = bacc.Bacc("TRN2", target_bir_lowering=False)

    x2T_d = nc.dram_tensor("x2T", [P, 2, N], BF16, kind="ExternalInput")
    x2_d = nc.dram_tensor("x2", [P, KCH, C], F32R, kind="ExternalInput")
    wqT_d = nc.dram_tensor("wqT", [P, 2, C8], BF16, kind="ExternalInput")
    wkT_d = nc.dram_tensor("wkT", [P, 2, C8], BF16, kind="ExternalInput")
    wvT_d = nc.dram_tensor("wvT", [P, 2, C + 2], BF16, kind="ExternalInput")
    bq_d = nc.dram_tensor("bqr", [1, C8], BF16, kind="ExternalInput")
    bk_d = nc.dram_tensor("bkr", [1, C8], BF16, kind="ExternalInput")
    bv_d = nc.dram_tensor("bvr", [1, C + 2], BF16, kind="ExternalInput")
    gam_d = nc.dram_tensor("gam", [1, 1], F32, kind="ExternalInput")
    ones_d = nc.dram_tensor("ones", [1, 512], BF16, kind="ExternalInput")
    out_d = nc.dram_tensor("out_rows", [NBLK, P, C], F32, kind="ExternalOutput")

    with tile.TileContext(nc) as tc:
        with (
            tc.tile_pool(name="singles", bufs=1) as singles,
            tc.tile_pool(name="persist", bufs=1) as persist,
            tc.tile_pool(name="pTp", bufs=3) as pTp,
            tc.tile_pool(name="sbout", bufs=3) as sbout,
            tc.tile_pool(name="small", bufs=10) as small,
            tc.tile_pool(name="psS", bufs=2, space="PSUM") as psS,
            tc.tile_pool(name="psO", bufs=4, space="PSUM") as psO,
            tc.tile_pool(name="psC", bufs=2, space="PSUM") as psC,
        ):
            # ---------------- Phase A: loads & constants ----------------
            x2T = persist.tile([P, 2, N], BF16, tag="x2T")
            nc.sync.dma_start(x2T[:], x2T_d.ap())
            x2 = persist.tile([P, KCH, C], F32R, tag="x2")
            nc.sync.dma_start(x2[:], x2_d.ap())
            wqT = singles.tile([P, 2, C8], BF16, tag="wqT")
            nc.sync.dma_start(wqT[:], wqT_d.ap())
            wkT = singles.tile([P, 2, C8], BF16, tag="wkT")
            nc.sync.dma_start(wkT[:], wkT_d.ap())
            wvT = singles.tile([P, 2, C + 2], BF16, tag="wvT")
            nc.sync.dma_start(wvT[:], wvT_d.ap())
            bqr = singles.tile([1, C8], BF16, tag="bqr")
            nc.sync.dma_start(bqr[:], bq_d.ap())
            bkr = singles.tile([1, C8], BF16, tag="bkr")
            nc.sync.dma_start(bkr[:], bk_d.ap())
            bvr = singles.tile([1, C + 2], BF16, tag="bvr")
            nc.sync.dma_start(bvr[:], bv_d.ap())
            ones = singles.tile([1, 512], BF16, tag="ones")
            nc.sync.dma_start(ones[:], ones_d.ap())
            ident = singles.tile([P, P], F32, tag="ident")
            make_identity(nc, ident[:])
            gb = singles.tile([P, 1], F32, tag="gb")
            nc.sync.dma_start(gb[:], gam_d.ap().to_broadcast([P, 1]))
            gh = singles.tile([P, 1], F32, tag="gh")
            nc.vector.tensor_scalar_mul(gh[:], gb[:], 0.5)

            # ------- Phase B: channel attention (E = X^T X, softmax) -------
            # E' = x2^T x2 = 4E ; A = exp(-(E - min)) / sum = exp(-.25 E' + .25 min')
            attn_n = singles.tile([P, 2, C], F32, tag="attn_n")
            attnTg = persist.tile([P, 2, C], BF16, tag="attnTg")
            for cb in range(2):
                e_ps = psO.tile([P, 512], F32, tag="o", name=f"e_{cb}")[:, :C]
                for nk in range(KCH):
                    nc.tensor.matmul(
                        e_ps,
                        x2[:, nk, cb * P : (cb + 1) * P],
                        x2[:, nk, :],
                        start=(nk == 0),
                        stop=(nk == KCH - 1),
                    )
                emin = small.tile([P, 1], F32, tag="sm", name=f"emin{cb}")
                nc.vector.tensor_reduce(
                    emin[:], e_ps, axis=AX.X, op=ALU.min
                )
                emq = small.tile([P, 1], F32, tag="sm", name=f"emq{cb}")
                nc.vector.tensor_scalar_mul(emq[:], emin[:], 0.25)
                us = small.tile([P, 1], F32, tag="sm", name=f"us{cb}")
                nc.scalar.activation(
                    attn_n[:, cb, :],
                    e_ps,
                    ACTF.Exp,
                    bias=emq[:],
                    scale=-0.25,
                    accum_out=us[:],
                )
                rc = small.tile([P, 1], F32, tag="sm", name=f"rc{cb}")
                nc.vector.reciprocal(rc[:], us[:])
                rcg = small.tile([P, 1], F32, tag="sm", name=f"rcg{cb}")
                nc.vector.tensor_mul(rcg[:], rc[:], gh[:])
                nc.vector.tensor_scalar_mul(
                    attn_n[:, cb, :], attn_n[:, cb, :], rcg[:]
                )
            for dd in range(2):
                for cc in range(2):
                    t_ps = psO.tile([P, 512], F32, tag="o", name=f"t_{dd}{cc}")[
                        :, :P
                    ]
                    nc.tensor.transpose(
                        t_ps, attn_n[:, cc, dd * P : (dd + 1) * P], ident[:]
                    )
                    nc.vector.tensor_copy(
                        attnTg[:, dd, cc * P : (cc + 1) * P], t_ps
                    )

            # ---------------- Phase C: kT, qT, V ----------------
            kT = persist.tile([C8, N], BF16, tag="kT")
            qT = persist.tile([C8, HALF], BF16, tag="qT")
            for seg in range(N // 512):
                kps = psS.tile([P, 512], F32, tag="s", name=f"k_{seg}")[:C8, :]
                for cc in range(2):
                    nc.tensor.matmul(
                        kps,
                        wkT[:, cc, :],
                        x2T[:, cc, seg * 512 : (seg + 1) * 512],
                        start=(cc == 0),
                        stop=False,
                    )
                nc.tensor.matmul(kps, bkr[:], ones[:], start=False, stop=True)
                nc.vector.tensor_copy(kT[:, seg * 512 : (seg + 1) * 512], kps)
            for seg in range(HALF // 512):
                qps = psS.tile([P, 512], F32, tag="s", name=f"q_{seg}")[:C8, :]
                for cc in range(2):
                    nc.tensor.matmul(
                        qps,
                        wqT[:, cc, :],
                        x2T[:, cc, seg * 512 : (seg + 1) * 512],
                        start=(cc == 0),
                        stop=False,
                    )
                nc.tensor.matmul(qps, bqr[:], ones[:], start=False, stop=True)
                nc.vector.tensor_copy(qT[:, seg * 512 : (seg + 1) * 512], qps)
            vaug = persist.tile([P, KCH, C + 2], BF16, tag="vaug")
            for nk in range(KCH):
                vps = psS.tile([P, 512], F32, tag="s", name=f"v_{nk}")[:, : C + 2]
                for cc in range(2):
                    nc.tensor.matmul(
                        vps,
                        x2T[:, cc, nk * P : (nk + 1) * P],
                        wvT[:, cc, :],
                        start=(cc == 0),
                        stop=False,
                    )
                nc.tensor.matmul(
                    vps, ones[:, :P], bvr[:], start=False, stop=True
                )
                nc.vector.tensor_copy(vaug[:, nk, :], vps)

            # ---------------- Phase D: point attention ----------------
            for m in range(HALF // 512):
                o_ps = [
                    psO.tile([P, 512], F32, tag="o", name=f"o_{m}_{j}")[:, : C + 2]
                    for j in range(4)
                ]
                for kk in range(KCH):
                    s_ps = psS.tile([P, 512], F32, tag="s", name=f"s_{m}_{kk}")
                    nc.tensor.matmul(
                        s_ps[:],
                        kT[:, kk * P : (kk + 1) * P],
                        qT[:, m * 512 : (m + 1) * 512],
                        start=True,
                        stop=True,
                    )
                    pT = pTp.tile([P, 512], BF16, tag="pT", name=f"p_{m}_{kk}")
                    nc.scalar.activation(pT[:], s_ps[:], ACTF.Exp)
                    for j in range(4):
                        nc.tensor.matmul(
                            o_ps[j],
                            pT[:, j * P : (j + 1) * P],
                            vaug[:, kk, :],
                            start=(kk == 0),
                            stop=(kk == KCH - 1),
                        )
                for j in range(4):
                    blk = m * 4 + j
                    c_ps = psC.tile([P, 512], F32, tag="c", name=f"c_{blk}")[
                        :, :C
                    ]
                    for dd in range(2):
                        nc.tensor.matmul(
                            c_ps,
                            x2T[:, dd, blk * P : (blk + 1) * P],
                            attnTg[:, dd, :],
                            start=(dd == 0),
                            stop=(dd == 1),
                        )
                    sq = small.tile([P, 1], F32, tag="sm", name=f"sq{blk}")
                    nc.vector.tensor_copy(sq[:], o_ps[j][:, C : C + 1])
                    rq = small.tile([P, 1], F32, tag="sm", name=f"rq{blk}")
                    nc.vector.reciprocal(rq[:], sq[:])
                    rqg = small.tile([P, 1], F32, tag="sm", name=f"rqg{blk}")
                    nc.vector.tensor_mul(rqg[:], rq[:], gb[:])
                    acc = sbout.tile([P, C], F32, tag="acc", name=f"acc{blk}")
                    nc.vector.tensor_scalar_mul(acc[:], o_ps[j][:, :C], rqg[:])
                    nc.vector.tensor_add(acc[:], acc[:], c_ps)
                    nc.vector.tensor_add(
                        acc[:], acc[:], x2[:, blk, :].bitcast(F32)
                    )
                    nc.sync.dma_start(out_d.ap()[blk], acc[:])

    nc.compile()
    return nc


def _prep_core_inputs(points, Wq, bq, Wk, bk, Wv, bv, gamma, core):
    b, h = core // 2, core % 2
    xb = np.asarray(points[b], dtype=np.float32)
    # own rows first, then the other half (key order is softmax-invariant
    # as long as kT and v use the same order, which they do)
    xp = np.concatenate([xb[h * HALF : (h + 1) * HALF], xb[(1 - h) * HALF : (2 - h) * HALF]])
    x2 = (2.0 * xp).reshape(KCH, P, C).transpose(1, 0, 2).copy()  # (128, 32, 256)
    x2T = np.ascontiguousarray(
        (2.0 * xp).T.reshape(2, P, N).transpose(1, 0, 2)
    ).astype(ml_dtypes.bfloat16)  # (128, 2, 4096)
    return {"x2T": x2T, "x2": x2}


def _prep_shared_inputs(Wq, bq, Wk, bk, Wv, bv, gamma):
    wqT = np.ascontiguousarray((0.5 * np.asarray(Wq, np.float32).T).reshape(2, P, C8).transpose(1, 0, 2)).astype(ml_dtypes.bfloat16)
    wkT = np.ascontiguousarray((0.5 * np.asarray(Wk, np.float32).T).reshape(2, P, C8).transpose(1, 0, 2)).astype(ml_dtypes.bfloat16)
    wvT_full = 0.5 * np.asarray(Wv, np.float32).T  # (256, 256)
    wvT_aug = np.concatenate(
        [wvT_full, np.zeros((C, 2), np.float32)], axis=1
    )  # (256, 258)
    wvT = np.ascontiguousarray(wvT_aug.reshape(2, P, C + 2).transpose(1, 0, 2)).astype(ml_dtypes.bfloat16)
    bv_aug = np.concatenate([np.asarray(bv, np.float32), [1.0, 0.0]]).reshape(1, C + 2)
    return {
        "wqT": wqT,
        "wkT": wkT,
        "wvT": wvT,
        "bqr": np.asarray(bq, np.float32).reshape(1, C8).astype(ml_dtypes.bfloat16),
        "bkr": np.asarray(bk, np.float32).reshape(1, C8).astype(ml_dtypes.bfloat16),
        "bvr": bv_aug.astype(ml_dtypes.bfloat16),
        "gam": np.asarray(gamma, np.float32).reshape(1, 1),
        "ones": np.ones((1, 512), ml_dtypes.bfloat16),
    }


def kernel(points, Wq, bq, Wk, bk, Wv, bv, gamma, **run_kwargs):
    if "nc" not in _CACHE:
        _CACHE["nc"] = _build_nc()
    nc = _CACHE["nc"]

    shared = _prep_shared_inputs(Wq, bq, Wk, bk, Wv, bv, gamma)
    in_maps = []
    for core in range(NCORES):
        m = dict(shared)
        m.update(_prep_core_inputs(points, Wq, bq, Wk, bk, Wv, bv, gamma, core))
        in_maps.append(m)

    res = run_bass_kernel_spmd(
        nc, in_maps, core_ids=list(range(NCORES)), **run_kwargs
    )
    out = np.empty((B, N, C), dtype=np.float32)
    for core in range(NCORES):
        b, h = core // 2, core % 2
        out[b, h * HALF : (h + 1) * HALF] = (
            res.results[core]["out_rows"].reshape(HALF, C)
        )
    if run_kwargs:
        kernel.last_results = res  # expose profile info to test harness
    return out


# revision 11
# speedup vs baseline: 1.2562x; 1.0339x over previous
"""DualAttention Trainium2 kernel (nn_DualAttention_44341242364496).

Reference math (per batch element, X = points[b], shape (N=4096, C=256)):
  q = X Wq^T + bq ; k = X Wk^T + bk          (N, 32)
  P = softmax(q k^T, axis=-1)                (N, N)
  v = X Wv^T + bv                            (N, 256)
  out_p = gamma * P v + X
  E = X X^T contracted over rows -> (C, C):  E = X^T X
  A = softmax(max_d(E) - E, axis=-1) == stable softmax(-E)
  out_c = gamma * A X^T' (einsum bcd,bnd->bnc) + X
  out = out_p + out_c = gamma*(Pv) + gamma*(X A^T) + 2X

Distribution: 8 cores; core c handles batch b=c//2, query-row half h=c%2.
Each core receives its batch's X in two layouts (scaled by 2, with the
core's own rows ordered first), computes k/v/E for the full batch element
(duplicated with its pair core), and produces its 2048 output rows.

Softmax tricks used on-chip:
 - point branch: exp() without max subtraction (scores are O(+-40), safe in
   fp32), denominator obtained by appending a ones-column to v so the PV
   matmul also yields row sums; division folded into the output scaling.
 - channel branch: softmax(max-E) == exp(-(E - min_row)) / sum.
All matmuls run in float32r (TF32-like PE fast path, 1 cycle/row).
"""

import sys

sys.path.insert(0, "/opt/trn_rl_repo")

import numpy as np
import ml_dtypes

import concourse.bass as bass  # noqa: F401  (bass types used via bacc/tile)
import concourse.mybir as mybir
import concourse.tile as tile
from concourse import bacc
from concourse.bass_utils import run_bass_kernel_spmd
from concourse.masks import make_identity

B, N, C = 4, 4096, 256
C8 = C // 8  # 32
NCORES = 8
HALF = N // 2  # 2048 query rows per core
NBLK = HALF // 128  # 16 output row blocks per core
KCH = N // 128  # 32 key chunks
P = 128

F32 = mybir.dt.float32
F32R = mybir.dt.float32r
BF16 = mybir.dt.bfloat16
AX = mybir.AxisListType
ALU = mybir.AluOpType
ACTF = mybir.ActivationFunctionType

_CACHE: dict = {}


def _build_nc():
    nc = bacc.Bacc("TRN2", target_bir_lowering=False)

    x2T_d = nc.dram_tensor("x2T", [P, 2, N], BF16, kind="ExternalInput")
    x2_d = nc.dram_tensor("x2", [P, KCH, C], F32R, kind="ExternalInput")
    wqT_d = nc.dram_tensor("wqT", [P, 2, C8], BF16, kind="ExternalInput")
    wkT_d = nc.dram_tensor("wkT", [P, 2, C8], BF16, kind="ExternalInput")
    wvT_d = nc.dram_tensor("wvT", [P, 2, C + 2], BF16, kind="ExternalInput")
    bq_d = nc.dram_tensor("bqr", [1, C8], BF16, kind="ExternalInput")
    bk_d = nc.dram_tensor("bkr", [1, C8], BF16, kind="ExternalInput")
    bv_d = nc.dram_tensor("bvr", [1, C + 2], BF16, kind="ExternalInput")
    gam_d = nc.dram_tensor("gam", [1, 1], F32, kind="ExternalInput")
    ones_d = nc.dram_tensor("ones", [1, 512], BF16, kind="ExternalInput")
    out_d = nc.dram_tensor("out_rows", [NBLK, P, C], F32, kind="ExternalOutput")

    with tile.TileContext(nc) as tc:
        with (
            tc.tile_pool(name="singles", bufs=1) as singles,
            tc.tile_pool(name="persist", bufs=1) as persist,
            tc.tile_pool(name="pTp", bufs=3) as pTp,
            tc.tile_pool(name="sbout", bufs=3) as sbout,
            tc.tile_pool(name="small", bufs=10) as small,
            tc.tile_pool(name="psS", bufs=2, space="PSUM") as psS,
            tc.tile_pool(name="psO", bufs=4, space="PSUM") as psO,
        ):
            # ---------------- Phase A: loads & constants ----------------
            x2T = persist.tile([P, 2, N], BF16, tag="x2T")
            nc.sync.dma_start(x2T[:], x2T_d.ap())
            x2 = persist.tile([P, KCH, C], F32R, tag="x2")
            nc.sync.dma_start(x2[:], x2_d.ap())
            wqT = singles.tile([P, 2, C8], BF16, tag="wqT")
            nc.sync.dma_start(wqT[:], wqT_d.ap())
            wkT = singles.tile([P, 2, C8], BF16, tag="wkT")
            nc.sync.dma_start(wkT[:], wkT_d.ap())
            wvT = singles.tile([P, 2, C + 2], BF16, tag="wvT")
            nc.sync.dma_start(wvT[:], wvT_d.ap())
            bqr = singles.tile([1, C8], BF16, tag="bqr")
            nc.sync.dma_start(bqr[:], bq_d.ap())
            bkr = singles.tile([1, C8], BF16, tag="bkr")
            nc.sync.dma_start(bkr[:], bk_d.ap())
            bvr = singles.tile([1, C + 2], BF16, tag="bvr")
            nc.sync.dma_start(bvr[:], bv_d.ap())
            ones = singles.tile([1, 512], BF16, tag="ones")
            nc.sync.dma_start(ones[:], ones_d.ap())
            ident = singles.tile([P, P], F32, tag="ident")
            make_identity(nc, ident[:])
            gb = singles.tile([P, 1], F32, tag="gb")
            nc.sync.dma_start(gb[:], gam_d.ap().to_broadcast([P, 1]))
            gh = singles.tile([P, 1], F32, tag="gh")
            nc.vector.tensor_scalar_mul(gh[:], gb[:], 0.5)

            # ------- Phase B: channel attention (E = X^T X, softmax) -------
            # E' = x2^T x2 = 4E ; A = exp(-(E - min)) / sum = exp(-.25 E' + .25 min')
            attn_n = singles.tile([P, 2, C], F32, tag="attn_n")
            attnTg = persist.tile([P, 2, C], BF16, tag="attnTg")
            for cb in range(2):
                e_ps = psO.tile([P, 512], F32, tag="o", name=f"e_{cb}")[:, :C]
                for nk in range(KCH):
                    nc.tensor.matmul(
                        e_ps,
                        x2[:, nk, cb * P : (cb + 1) * P],
                        x2[:, nk, :],
                        start=(nk == 0),
                        stop=(nk == KCH - 1),
                    )
                emin = small.tile([P, 1], F32, tag="sm", name=f"emin{cb}")
                nc.vector.tensor_reduce(
                    emin[:], e_ps, axis=AX.X, op=ALU.min
                )
                emq = small.tile([P, 1], F32, tag="sm", name=f"emq{cb}")
                nc.vector.tensor_scalar_mul(emq[:], emin[:], 0.25)
                us = small.tile([P, 1], F32, tag="sm", name=f"us{cb}")
                nc.scalar.activation(
                    attn_n[:, cb, :],
                    e_ps,
                    ACTF.Exp,
                    bias=emq[:],
                    scale=-0.25,
                    accum_out=us[:],
                )
                rc = small.tile([P, 1], F32, tag="sm", name=f"rc{cb}")
                nc.vector.reciprocal(rc[:], us[:])
                rcg = small.tile([P, 1], F32, tag="sm", name=f"rcg{cb}")
                nc.vector.tensor_mul(rcg[:], rc[:], gh[:])
                nc.vector.tensor_scalar_mul(
                    attn_n[:, cb, :], attn_n[:, cb, :], rcg[:]
                )
            for dd in range(2):
                for cc in range(2):
                    t_ps = psO.tile([P, 512], F32, tag="o", name=f"t_{dd}{cc}")[
                        :, :P
                    ]
                    nc.tensor.transpose(
                        t_ps, attn_n[:, cc, dd * P : (dd + 1) * P], ident[:]
                    )
                    nc.vector.tensor_copy(
                        attnTg[:, dd, cc * P : (cc + 1) * P], t_ps
                    )

            # ---------------- Phase C: kT, qT, V ----------------
            kT = persist.tile([C8, N], BF16, tag="kT")
            qT = persist.tile([C8, HALF], BF16, tag="qT")
            for seg in range(N // 512):
                kps = psS.tile([P, 1024], F32, tag="s", name=f"k_{seg}")[:C8, :512]
                for cc in range(2):
                    nc.tensor.matmul(
                        kps,
                        wkT[:, cc, :],
                        x2T[:, cc, seg * 512 : (seg + 1) * 512],
                        start=(cc == 0),
                        stop=False,
                    )
                nc.tensor.matmul(kps, bkr[:], ones[:], start=False, stop=True)
                nc.vector.tensor_copy(kT[:, seg * 512 : (seg + 1) * 512], kps)
            for seg in range(HALF // 512):
                qps = psS.tile([P, 1024], F32, tag="s", name=f"q_{seg}")[:C8, :512]
                for cc in range(2):
                    nc.tensor.matmul(
                        qps,
                        wqT[:, cc, :],
                        x2T[:, cc, seg * 512 : (seg + 1) * 512],
                        start=(cc == 0),
                        stop=False,
                    )
                nc.tensor.matmul(qps, bqr[:], ones[:], start=False, stop=True)
                nc.vector.tensor_copy(qT[:, seg * 512 : (seg + 1) * 512], qps)
            vaug = persist.tile([P, KCH, C + 2], BF16, tag="vaug")
            for nk in range(KCH):
                vps = psS.tile([P, 1024], F32, tag="s", name=f"v_{nk}")[:, : C + 2]
                for cc in range(2):
                    nc.tensor.matmul(
                        vps,
                        x2T[:, cc, nk * P : (nk + 1) * P],
                        wvT[:, cc, :],
                        start=(cc == 0),
                        stop=False,
                    )
                nc.tensor.matmul(
                    vps, ones[:, :P], bvr[:], start=False, stop=True
                )
                nc.vector.tensor_copy(vaug[:, nk, :], vps)

            # ---- channel-branch output rows, precomputed into SBUF ----
            # outc_sb[blk] = gamma/2 * attn_c @ x2_rows(blk) + x2_rows(blk)
            # (residual folded in here so the main-loop epilogue is 2 DVE ops)
            outc_sb = persist.tile([P, NBLK, C], F32, tag="outc_sb")
            for blk in range(NBLK):
                c_ps = psS.tile([P, 1024], F32, tag="s", name=f"c_{blk}")[:, :C]
                for dd in range(2):
                    nc.tensor.matmul(
                        c_ps,
                        x2T[:, dd, blk * P : (blk + 1) * P],
                        attnTg[:, dd, :],
                        start=(dd == 0),
                        stop=(dd == 1),
                    )
                nc.vector.tensor_add(
                    outc_sb[:, blk, :], c_ps, x2[:, blk, :].bitcast(F32)
                )

            # ---------------- Phase D: point attention ----------------
            # 16 rounds per macro block, each covering 2 key chunks with one
            # double-width (2-bank) scores psum + a single wide exp.
            for m in range(HALF // 512):
                o_ps = [
                    psO.tile([P, 512], F32, tag="o", name=f"o_{m}_{j}")[:, : C + 2]
                    for j in range(4)
                ]
                for r in range(KCH // 2):
                    s_ps = psS.tile([P, 1024], F32, tag="s", name=f"s_{m}_{r}")
                    for half in range(2):
                        kk = 2 * r + half
                        nc.tensor.matmul(
                            s_ps[:, half * 512 : (half + 1) * 512],
                            kT[:, kk * P : (kk + 1) * P],
                            qT[:, m * 512 : (m + 1) * 512],
                            start=True,
                            stop=True,
                        )
                    pT = pTp.tile([P, 1024], BF16, tag="pT", name=f"p_{m}_{r}")
                    nc.scalar.activation(pT[:], s_ps[:], ACTF.Exp)
                    for half in range(2):
                        kk = 2 * r + half
                        for j in range(4):
                            nc.tensor.matmul(
                                o_ps[j],
                                pT[:, half * 512 + j * P : half * 512 + (j + 1) * P],
                                vaug[:, kk, :],
                                start=(kk == 0),
                                stop=(kk == KCH - 1),
                            )
                for j in range(4):
                    blk = m * 4 + j
                    sq = small.tile([P, 1], F32, tag="sm", name=f"sq{blk}")
                    nc.vector.tensor_copy(sq[:], o_ps[j][:, C : C + 1])
                    rq = small.tile([P, 1], F32, tag="sm", name=f"rq{blk}")
                    nc.vector.reciprocal(rq[:], sq[:])
                    rqg = small.tile([P, 1], F32, tag="sm", name=f"rqg{blk}")
                    nc.vector.tensor_mul(rqg[:], rq[:], gb[:])
                    acc = sbout.tile([P, C], F32, tag="acc", name=f"acc{blk}")
                    nc.vector.tensor_scalar_mul(acc[:], o_ps[j][:, :C], rqg[:])
                    nc.vector.tensor_add(acc[:], acc[:], outc_sb[:, blk, :])
                    nc.sync.dma_start(out_d.ap()[blk], acc[:])

    nc.compile()
    return nc


def _prep_core_inputs(points, Wq, bq, Wk, bk, Wv, bv, gamma, core):
    b, h = core // 2, core % 2
    xb = np.asarray(points[b], dtype=np.float32)
    # own rows first, then the other half (key order is softmax-invariant
    # as long as kT and v use the same order, which they do)
    xp = np.concatenate([xb[h * HALF : (h + 1) * HALF], xb[(1 - h) * HALF : (2 - h) * HALF]])
    x2 = (2.0 * xp).reshape(KCH, P, C).transpose(1, 0, 2).copy()  # (128, 32, 256)
    x2T = np.ascontiguousarray(
        (2.0 * xp).T.reshape(2, P, N).transpose(1, 0, 2)
    ).astype(ml_dtypes.bfloat16)  # (128, 2, 4096)
    return {"x2T": x2T, "x2": x2}


def _prep_shared_inputs(Wq, bq, Wk, bk, Wv, bv, gamma):
    wqT = np.ascontiguousarray((0.5 * np.asarray(Wq, np.float32).T).reshape(2, P, C8).transpose(1, 0, 2)).astype(ml_dtypes.bfloat16)
    wkT = np.ascontiguousarray((0.5 * np.asarray(Wk, np.float32).T).reshape(2, P, C8).transpose(1, 0, 2)).astype(ml_dtypes.bfloat16)
    wvT_full = 0.5 * np.asarray(Wv, np.float32).T  # (256, 256)
    wvT_aug = np.concatenate(
        [wvT_full, np.zeros((C, 2), np.float32)], axis=1
    )  # (256, 258)
    wvT = np.ascontiguousarray(wvT_aug.reshape(2, P, C + 2).transpose(1, 0, 2)).astype(ml_dtypes.bfloat16)
    bv_aug = np.concatenate([np.asarray(bv, np.float32), [1.0, 0.0]]).reshape(1, C + 2)
    return {
        "wqT": wqT,
        "wkT": wkT,
        "wvT": wvT,
        "bqr": np.asarray(bq, np.float32).reshape(1, C8).astype(ml_dtypes.bfloat16),
        "bkr": np.asarray(bk, np.float32).reshape(1, C8).astype(ml_dtypes.bfloat16),
        "bvr": bv_aug.astype(ml_dtypes.bfloat16),
        "gam": np.asarray(gamma, np.float32).reshape(1, 1),
        "ones": np.ones((1, 512), ml_dtypes.bfloat16),
    }


def kernel(points, Wq, bq, Wk, bk, Wv, bv, gamma, **run_kwargs):
    if "nc" not in _CACHE:
        _CACHE["nc"] = _build_nc()
    nc = _CACHE["nc"]

    shared = _prep_shared_inputs(Wq, bq, Wk, bk, Wv, bv, gamma)
    in_maps = []
    for core in range(NCORES):
        m = dict(shared)
        m.update(_prep_core_inputs(points, Wq, bq, Wk, bk, Wv, bv, gamma, core))
        in_maps.append(m)

    res = run_bass_kernel_spmd(
        nc, in_maps, core_ids=list(range(NCORES)), **run_kwargs
    )
    out = np.empty((B, N, C), dtype=np.float32)
    for core in range(NCORES):
        b, h = core // 2, core % 2
        out[b, h * HALF : (h + 1) * HALF] = (
            res.results[core]["out_rows"].reshape(HALF, C)
        )
    if run_kwargs:
        kernel.last_results = res  # expose profile info to test harness
    return out


# revision 18
# speedup vs baseline: 1.4034x; 1.1172x over previous
"""DualAttention Trainium2 kernel (nn_DualAttention_44341242364496).

Reference math (per batch element, X = points[b], shape (N=4096, C=256)):
  q = X Wq^T + bq ; k = X Wk^T + bk          (N, 32)
  P = softmax(q k^T, axis=-1)                (N, N)
  v = X Wv^T + bv                            (N, 256)
  out_p = gamma * P v + X
  E = X X^T contracted over rows -> (C, C):  E = X^T X
  A = softmax(max_d(E) - E, axis=-1) == stable softmax(-E)
  out_c = gamma * A X^T' (einsum bcd,bnd->bnc) + X
  out = out_p + out_c = gamma*(Pv) + gamma*(X A^T) + 2X

Distribution: 8 cores; core c handles batch b=c//2, query-row half h=c%2.
Each core receives its batch's X in two layouts (scaled by 2, with the
core's own rows ordered first), computes k/v/E for the full batch element
(duplicated with its pair core), and produces its 2048 output rows.

Softmax tricks used on-chip:
 - point branch: exp() without max subtraction (scores are O(+-40), safe in
   fp32), denominator obtained by appending a ones-column to v so the PV
   matmul also yields row sums; division folded into the output scaling.
 - channel branch: softmax(max-E) == exp(-(E - min_row)) / sum.
All matmuls run in float32r (TF32-like PE fast path, 1 cycle/row).
"""

import sys

sys.path.insert(0, "/opt/trn_rl_repo")

import numpy as np
import ml_dtypes

import concourse.bass as bass  # noqa: F401  (bass types used via bacc/tile)
import concourse.mybir as mybir
import concourse.tile as tile
from concourse import bacc
from concourse.bass_utils import run_bass_kernel_spmd
from concourse.masks import make_identity

B, N, C = 4, 4096, 256
C8 = C // 8  # 32
NCORES = 8
HALF = N // 2  # 2048 query rows per core
NBLK = HALF // 128  # 16 output row blocks per core
KCH = N // 128  # 32 key chunks
P = 128

F32 = mybir.dt.float32
F32R = mybir.dt.float32r
BF16 = mybir.dt.bfloat16
AX = mybir.AxisListType
ALU = mybir.AluOpType
ACTF = mybir.ActivationFunctionType

_CACHE: dict = {}


def _build_nc():
    nc = bacc.Bacc("TRN2", target_bir_lowering=False)

    x2T_d = nc.dram_tensor("x2T", [P, 2, N], BF16, kind="ExternalInput")
    x2_d = nc.dram_tensor("x2", [P, KCH, C], F32R, kind="ExternalInput")
    wqT_d = nc.dram_tensor("wqT", [P, 2, C8], BF16, kind="ExternalInput")
    wkT_d = nc.dram_tensor("wkT", [P, 2, C8], BF16, kind="ExternalInput")
    wvT_d = nc.dram_tensor("wvT", [P, 2, C + 2], BF16, kind="ExternalInput")
    bq_d = nc.dram_tensor("bqc", [C8, 1], F32, kind="ExternalInput")
    bk_d = nc.dram_tensor("bkc", [C8, 1], F32, kind="ExternalInput")
    bv_d = nc.dram_tensor("bvr", [1, C + 2], F32, kind="ExternalInput")
    gam_d = nc.dram_tensor("gam", [1, 1], F32, kind="ExternalInput")
    out_d = nc.dram_tensor("out_rows", [NBLK, P, C], F32, kind="ExternalOutput")

    with tile.TileContext(nc) as tc:
        with (
            tc.tile_pool(name="singles", bufs=1) as singles,
            tc.tile_pool(name="persist", bufs=1) as persist,
            tc.tile_pool(name="pTp", bufs=3) as pTp,
            tc.tile_pool(name="sbout", bufs=3) as sbout,
            tc.tile_pool(name="small", bufs=10) as small,
            tc.tile_pool(name="psS", bufs=2, space="PSUM") as psS,
            tc.tile_pool(name="psO", bufs=4, space="PSUM") as psO,
        ):
            # ---------------- Phase A: loads & constants ----------------
            # x2 is split into chunk-group DMAs so energy matmuls can start
            # as soon as the first chunks land; weights go on the ACT HWDGE
            # queue so they don't queue behind the big SP transfers.
            x2 = persist.tile([P, KCH, C], F32R, tag="x2")
            for g in range(8):
                nc.sync.dma_start(
                    x2[:, g * 4 : (g + 1) * 4, :], x2_d.ap()[:, g * 4 : (g + 1) * 4, :]
                )
            x2T = persist.tile([P, 2, N], BF16, tag="x2T")
            for g in range(2):
                nc.sync.dma_start(x2T[:, g, :], x2T_d.ap()[:, g, :])
            wqT = singles.tile([P, 2, C8], BF16, tag="wqT")
            nc.scalar.dma_start(wqT[:], wqT_d.ap())
            wkT = singles.tile([P, 2, C8], BF16, tag="wkT")
            nc.scalar.dma_start(wkT[:], wkT_d.ap())
            wvT = singles.tile([P, 2, C + 2], BF16, tag="wvT")
            nc.scalar.dma_start(wvT[:], wvT_d.ap())
            bqc = singles.tile([C8, 1], F32, tag="bqc")
            nc.scalar.dma_start(bqc[:], bq_d.ap())
            bkc = singles.tile([C8, 1], F32, tag="bkc")
            nc.scalar.dma_start(bkc[:], bk_d.ap())
            bvb = singles.tile([P, C + 2], F32, tag="bvb")
            nc.gpsimd.dma_start(
                bvb[:],
                bass.AP(tensor=bv_d, offset=0, ap=[[0, P], [1, C + 2]]),
            )
            ident = singles.tile([P, P], F32, tag="ident")
            make_identity(nc, ident[:])
            gb = singles.tile([P, 1], F32, tag="gb")
            nc.scalar.dma_start(gb[:], gam_d.ap().to_broadcast([P, 1]))
            gh = singles.tile([P, 1], F32, tag="gh")
            nc.vector.tensor_scalar_mul(gh[:], gb[:], 0.5)

            # ------- Phase B: channel attention (E = X^T X, softmax) -------
            # E' = x2^T x2 = 4E ; A = exp(-(E - min)) / sum = exp(-.25 E' + .25 min')
            attn_n = singles.tile([P, 2, C], F32, tag="attn_n")
            attnTg = persist.tile([P, 2, C], BF16, tag="attnTg")
            for cb in range(2):
                e_ps = psO.tile([P, 512], F32, tag="o", name=f"e_{cb}")[:, :C]
                for nk in range(KCH):
                    nc.tensor.matmul(
                        e_ps,
                        x2[:, nk, cb * P : (cb + 1) * P],
                        x2[:, nk, :],
                        start=(nk == 0),
                        stop=(nk == KCH - 1),
                    )
                emin = small.tile([P, 1], F32, tag="sm", name=f"emin{cb}")
                nc.vector.tensor_reduce(
                    emin[:], e_ps, axis=AX.X, op=ALU.min
                )
                emq = small.tile([P, 1], F32, tag="sm", name=f"emq{cb}")
                nc.vector.tensor_scalar_mul(emq[:], emin[:], 0.25)
                us = small.tile([P, 1], F32, tag="sm", name=f"us{cb}")
                nc.scalar.activation(
                    attn_n[:, cb, :],
                    e_ps,
                    ACTF.Exp,
                    bias=emq[:],
                    scale=-0.25,
                    accum_out=us[:],
                )
                rc = small.tile([P, 1], F32, tag="sm", name=f"rc{cb}")
                nc.vector.reciprocal(rc[:], us[:])
                rcg = small.tile([P, 1], F32, tag="sm", name=f"rcg{cb}")
                nc.vector.tensor_mul(rcg[:], rc[:], gh[:])
                nc.vector.tensor_scalar_mul(
                    attn_n[:, cb, :], attn_n[:, cb, :], rcg[:]
                )
            for dd in range(2):
                for cc in range(2):
                    t_ps = psO.tile([P, 512], F32, tag="o", name=f"t_{dd}{cc}")[
                        :, :P
                    ]
                    nc.tensor.transpose(
                        t_ps, attn_n[:, cc, dd * P : (dd + 1) * P], ident[:]
                    )
                    nc.vector.tensor_copy(
                        attnTg[:, dd, cc * P : (cc + 1) * P], t_ps
                    )

            # ---------------- Phase C: kT, qT, V ----------------
            kT = persist.tile([C8, N], BF16, tag="kT")
            qT = persist.tile([C8, HALF], BF16, tag="qT")
            for seg in range(N // 512):
                kps = psS.tile([P, 1024], F32, tag="s", name=f"k_{seg}")[:C8, :512]
                for cc in range(2):
                    nc.tensor.matmul(
                        kps,
                        wkT[:, cc, :],
                        x2T[:, cc, seg * 512 : (seg + 1) * 512],
                        start=(cc == 0),
                        stop=(cc == 1),
                    )
                nc.vector.tensor_scalar_add(
                    kT[:, seg * 512 : (seg + 1) * 512], kps, bkc[:]
                )
            for seg in range(HALF // 512):
                qps = psS.tile([P, 1024], F32, tag="s", name=f"q_{seg}")[:C8, :512]
                for cc in range(2):
                    nc.tensor.matmul(
                        qps,
                        wqT[:, cc, :],
                        x2T[:, cc, seg * 512 : (seg + 1) * 512],
                        start=(cc == 0),
                        stop=(cc == 1),
                    )
                nc.vector.tensor_scalar_add(
                    qT[:, seg * 512 : (seg + 1) * 512], qps, bqc[:]
                )
            vaug = persist.tile([P, KCH, C + 2], BF16, tag="vaug")
            for nk in range(KCH):
                vps = psS.tile([P, 1024], F32, tag="s", name=f"v_{nk}")[:, : C + 2]
                for cc in range(2):
                    nc.tensor.matmul(
                        vps,
                        x2T[:, cc, nk * P : (nk + 1) * P],
                        wvT[:, cc, :],
                        start=(cc == 0),
                        stop=(cc == 1),
                    )
                nc.vector.tensor_add(vaug[:, nk, :], vps, bvb[:])

            # ---- channel-branch output rows, precomputed into SBUF ----
            # outc_sb[blk] = gamma/2 * attn_c @ x2_rows(blk) + x2_rows(blk)
            # (residual folded in here so the main-loop epilogue is 2 DVE ops)
            outc_sb = persist.tile([P, NBLK, C], F32, tag="outc_sb")
            for blk in range(NBLK):
                c_ps = psS.tile([P, 1024], F32, tag="s", name=f"c_{blk}")[:, :C]
                for dd in range(2):
                    nc.tensor.matmul(
                        c_ps,
                        x2T[:, dd, blk * P : (blk + 1) * P],
                        attnTg[:, dd, :],
                        start=(dd == 0),
                        stop=(dd == 1),
                    )
                nc.vector.tensor_add(
                    outc_sb[:, blk, :], c_ps, x2[:, blk, :].bitcast(F32)
                )

            # ---------------- Phase D: point attention ----------------
            # 16 rounds per macro block, each covering 2 key chunks with one
            # double-width (2-bank) scores psum + a single wide exp.
            for m in range(HALF // 512):
                o_ps = [
                    psO.tile([P, 512], F32, tag="o", name=f"o_{m}_{j}")[:, : C + 2]
                    for j in range(4)
                ]
                for r in range(KCH // 2):
                    s_ps = psS.tile([P, 1024], F32, tag="s", name=f"s_{m}_{r}")
                    for half in range(2):
                        kk = 2 * r + half
                        nc.tensor.matmul(
                            s_ps[:, half * 512 : (half + 1) * 512],
                            kT[:, kk * P : (kk + 1) * P],
                            qT[:, m * 512 : (m + 1) * 512],
                            start=True,
                            stop=True,
                        )
                    pT = pTp.tile([P, 1024], BF16, tag="pT", name=f"p_{m}_{r}")
                    nc.scalar.activation(pT[:], s_ps[:], ACTF.Exp)
                    for half in range(2):
                        kk = 2 * r + half
                        for j in range(4):
                            nc.tensor.matmul(
                                o_ps[j],
                                pT[:, half * 512 + j * P : half * 512 + (j + 1) * P],
                                vaug[:, kk, :],
                                start=(kk == 0),
                                stop=(kk == KCH - 1),
                            )
                for j in range(4):
                    blk = m * 4 + j
                    # single fast PSUM read frees the O bank for the next
                    # macro block's PV accumulation almost immediately
                    osb = sbout.tile([P, C + 2], F32, tag="osb", name=f"osb{blk}")
                    nc.vector.tensor_copy(osb[:], o_ps[j])
                    rq = small.tile([P, 1], F32, tag="sm", name=f"rq{blk}")
                    nc.vector.reciprocal(rq[:], osb[:, C : C + 1])
                    rqg = small.tile([P, 1], F32, tag="sm", name=f"rqg{blk}")
                    nc.vector.tensor_mul(rqg[:], rq[:], gb[:])
                    acc = sbout.tile([P, C], F32, tag="acc", name=f"acc{blk}")
                    nc.vector.tensor_scalar_mul(acc[:], osb[:, :C], rqg[:])
                    nc.vector.tensor_add(acc[:], acc[:], outc_sb[:, blk, :])
                    nc.sync.dma_start(out_d.ap()[blk], acc[:])

    nc.compile()
    return nc


def _prep_core_inputs(points, Wq, bq, Wk, bk, Wv, bv, gamma, core):
    b, h = core // 2, core % 2
    xb = np.asarray(points[b], dtype=np.float32)
    # own rows first, then the other half (key order is softmax-invariant
    # as long as kT and v use the same order, which they do)
    xp = np.concatenate([xb[h * HALF : (h + 1) * HALF], xb[(1 - h) * HALF : (2 - h) * HALF]])
    x2 = (2.0 * xp).reshape(KCH, P, C).transpose(1, 0, 2).copy()  # (128, 32, 256)
    x2T = np.ascontiguousarray(
        (2.0 * xp).T.reshape(2, P, N).transpose(1, 0, 2)
    ).astype(ml_dtypes.bfloat16)  # (128, 2, 4096)
    return {"x2T": x2T, "x2": x2}


def _prep_shared_inputs(Wq, bq, Wk, bk, Wv, bv, gamma):
    wqT = np.ascontiguousarray((0.5 * np.asarray(Wq, np.float32).T).reshape(2, P, C8).transpose(1, 0, 2)).astype(ml_dtypes.bfloat16)
    wkT = np.ascontiguousarray((0.5 * np.asarray(Wk, np.float32).T).reshape(2, P, C8).transpose(1, 0, 2)).astype(ml_dtypes.bfloat16)
    wvT_full = 0.5 * np.asarray(Wv, np.float32).T  # (256, 256)
    wvT_aug = np.concatenate(
        [wvT_full, np.zeros((C, 2), np.float32)], axis=1
    )  # (256, 258)
    wvT = np.ascontiguousarray(wvT_aug.reshape(2, P, C + 2).transpose(1, 0, 2)).astype(ml_dtypes.bfloat16)
    bv_aug = np.concatenate([np.asarray(bv, np.float32), [1.0, 0.0]]).reshape(1, C + 2)
    return {
        "wqT": wqT,
        "wkT": wkT,
        "wvT": wvT,
        "bqc": np.asarray(bq, np.float32).reshape(C8, 1),
        "bkc": np.asarray(bk, np.float32).reshape(C8, 1),
        "bvr": bv_aug,
        "gam": np.asarray(gamma, np.float32).reshape(1, 1),
    }


def kernel(points, Wq, bq, Wk, bk, Wv, bv, gamma, **run_kwargs):
    if "nc" not in _CACHE:
        _CACHE["nc"] = _build_nc()
    nc = _CACHE["nc"]

    shared = _prep_shared_inputs(Wq, bq, Wk, bk, Wv, bv, gamma)
    in_maps = []
    for core in range(NCORES):
        m = dict(shared)
        m.update(_prep_core_inputs(points, Wq, bq, Wk, bk, Wv, bv, gamma, core))
        in_maps.append(m)

    res = run_bass_kernel_spmd(
        nc, in_maps, core_ids=list(range(NCORES)), **run_kwargs
    )
    out = np.empty((B, N, C), dtype=np.float32)
    for core in range(NCORES):
        b, h = core // 2, core % 2
        out[b, h * HALF : (h + 1) * HALF] = (
            res.results[core]["out_rows"].reshape(HALF, C)
        )
    if run_kwargs:
        kernel.last_results = res  # expose profile info to test harness
    return out


# revision 19
# speedup vs baseline: 1.4552x; 1.0369x over previous
"""DualAttention Trainium2 kernel (nn_DualAttention_44341242364496).

Reference math (per batch element, X = points[b], shape (N=4096, C=256)):
  q = X Wq^T + bq ; k = X Wk^T + bk          (N, 32)
  P = softmax(q k^T, axis=-1)                (N, N)
  v = X Wv^T + bv                            (N, 256)
  out_p = gamma * P v + X
  E = X X^T contracted over rows -> (C, C):  E = X^T X
  A = softmax(max_d(E) - E, axis=-1) == stable softmax(-E)
  out_c = gamma * A X^T' (einsum bcd,bnd->bnc) + X
  out = out_p + out_c = gamma*(Pv) + gamma*(X A^T) + 2X

Distribution: 8 cores; core c handles batch b=c//2, query-row half h=c%2.
Each core receives its batch's X in two layouts (scaled by 2, with the
core's own rows ordered first), computes k/v/E for the full batch element
(duplicated with its pair core), and produces its 2048 output rows.

Softmax tricks used on-chip:
 - point branch: exp() without max subtraction (scores are O(+-40), safe in
   fp32), denominator obtained by appending a ones-column to v so the PV
   matmul also yields row sums; division folded into the output scaling.
 - channel branch: softmax(max-E) == exp(-(E - min_row)) / sum.
All matmuls run in float32r (TF32-like PE fast path, 1 cycle/row).
"""

import sys

sys.path.insert(0, "/opt/trn_rl_repo")

import numpy as np
import ml_dtypes

import concourse.bass as bass  # noqa: F401  (bass types used via bacc/tile)
import concourse.mybir as mybir
import concourse.tile as tile
from concourse import bacc
from concourse.bass_utils import run_bass_kernel_spmd
from concourse.masks import make_identity

B, N, C = 4, 4096, 256
C8 = C // 8  # 32
NCORES = 8
HALF = N // 2  # 2048 query rows per core
NBLK = HALF // 128  # 16 output row blocks per core
KCH = N // 128  # 32 key chunks
P = 128

F32 = mybir.dt.float32
F32R = mybir.dt.float32r
BF16 = mybir.dt.bfloat16
F16 = mybir.dt.float16
AX = mybir.AxisListType
ALU = mybir.AluOpType
ACTF = mybir.ActivationFunctionType

_CACHE: dict = {}


def _build_nc():
    nc = bacc.Bacc("TRN2", target_bir_lowering=False)

    x2T_d = nc.dram_tensor("x2T", [P, 2, N], F16, kind="ExternalInput")
    x2_d = nc.dram_tensor("x2", [P, KCH, C], F16, kind="ExternalInput")
    wqT_d = nc.dram_tensor("wqT", [P, 2, C8], F16, kind="ExternalInput")
    wkT_d = nc.dram_tensor("wkT", [P, 2, C8], F16, kind="ExternalInput")
    wvT_d = nc.dram_tensor("wvT", [P, 2, C + 2], F16, kind="ExternalInput")
    bq_d = nc.dram_tensor("bqc", [C8, 1], F32, kind="ExternalInput")
    bk_d = nc.dram_tensor("bkc", [C8, 1], F32, kind="ExternalInput")
    bv_d = nc.dram_tensor("bvr", [1, C + 2], F32, kind="ExternalInput")
    gam_d = nc.dram_tensor("gam", [1, 1], F32, kind="ExternalInput")
    out_d = nc.dram_tensor("out_rows", [NBLK, P, C], F32, kind="ExternalOutput")

    with tile.TileContext(nc) as tc:
        with (
            tc.tile_pool(name="singles", bufs=1) as singles,
            tc.tile_pool(name="persist", bufs=1) as persist,
            tc.tile_pool(name="pTp", bufs=3) as pTp,
            tc.tile_pool(name="sbout", bufs=3) as sbout,
            tc.tile_pool(name="small", bufs=10) as small,
            tc.tile_pool(name="psS", bufs=2, space="PSUM") as psS,
            tc.tile_pool(name="psO", bufs=4, space="PSUM") as psO,
        ):
            # ---------------- Phase A: loads & constants ----------------
            # x2 is split into chunk-group DMAs so energy matmuls can start
            # as soon as the first chunks land; weights go on the ACT HWDGE
            # queue so they don't queue behind the big SP transfers.
            x2 = persist.tile([P, KCH, C], F16, tag="x2")
            for g in range(8):
                nc.sync.dma_start(
                    x2[:, g * 4 : (g + 1) * 4, :], x2_d.ap()[:, g * 4 : (g + 1) * 4, :]
                )
            x2T = persist.tile([P, 2, N], F16, tag="x2T")
            for g in range(2):
                nc.sync.dma_start(x2T[:, g, :], x2T_d.ap()[:, g, :])
            wqT = singles.tile([P, 2, C8], F16, tag="wqT")
            nc.scalar.dma_start(wqT[:], wqT_d.ap())
            wkT = singles.tile([P, 2, C8], F16, tag="wkT")
            nc.scalar.dma_start(wkT[:], wkT_d.ap())
            wvT = singles.tile([P, 2, C + 2], F16, tag="wvT")
            nc.scalar.dma_start(wvT[:], wvT_d.ap())
            bqc = singles.tile([C8, 1], F32, tag="bqc")
            nc.scalar.dma_start(bqc[:], bq_d.ap())
            bkc = singles.tile([C8, 1], F32, tag="bkc")
            nc.scalar.dma_start(bkc[:], bk_d.ap())
            bvb = singles.tile([P, C + 2], F32, tag="bvb")
            nc.gpsimd.dma_start(
                bvb[:],
                bass.AP(tensor=bv_d, offset=0, ap=[[0, P], [1, C + 2]]),
            )
            ident = singles.tile([P, P], F32, tag="ident")
            make_identity(nc, ident[:])
            gb = singles.tile([P, 1], F32, tag="gb")
            nc.scalar.dma_start(gb[:], gam_d.ap().to_broadcast([P, 1]))
            gh = singles.tile([P, 1], F32, tag="gh")
            nc.vector.tensor_scalar_mul(gh[:], gb[:], 0.5)

            # ------- Phase B: channel attention (E = X^T X, softmax) -------
            # E' = x2^T x2 = 4E ; A = exp(-(E - min)) / sum = exp(-.25 E' + .25 min')
            attn_n = singles.tile([P, 2, C], F32, tag="attn_n")
            attnTg = persist.tile([P, 2, C], F16, tag="attnTg")
            for cb in range(2):
                e_ps = psO.tile([P, 512], F32, tag="o", name=f"e_{cb}")[:, :C]
                for nk in range(KCH):
                    nc.tensor.matmul(
                        e_ps,
                        x2[:, nk, cb * P : (cb + 1) * P],
                        x2[:, nk, :],
                        start=(nk == 0),
                        stop=(nk == KCH - 1),
                    )
                emin = small.tile([P, 1], F32, tag="sm", name=f"emin{cb}")
                nc.vector.tensor_reduce(
                    emin[:], e_ps, axis=AX.X, op=ALU.min
                )
                emq = small.tile([P, 1], F32, tag="sm", name=f"emq{cb}")
                nc.vector.tensor_scalar_mul(emq[:], emin[:], 0.25)
                us = small.tile([P, 1], F32, tag="sm", name=f"us{cb}")
                nc.scalar.activation(
                    attn_n[:, cb, :],
                    e_ps,
                    ACTF.Exp,
                    bias=emq[:],
                    scale=-0.25,
                    accum_out=us[:],
                )
                rc = small.tile([P, 1], F32, tag="sm", name=f"rc{cb}")
                nc.vector.reciprocal(rc[:], us[:])
                rcg = small.tile([P, 1], F32, tag="sm", name=f"rcg{cb}")
                nc.vector.tensor_mul(rcg[:], rc[:], gh[:])
                nc.vector.tensor_scalar_mul(
                    attn_n[:, cb, :], attn_n[:, cb, :], rcg[:]
                )
            for dd in range(2):
                for cc in range(2):
                    t_ps = psO.tile([P, 512], F32, tag="o", name=f"t_{dd}{cc}")[
                        :, :P
                    ]
                    nc.tensor.transpose(
                        t_ps, attn_n[:, cc, dd * P : (dd + 1) * P], ident[:]
                    )
                    nc.vector.tensor_copy(
                        attnTg[:, dd, cc * P : (cc + 1) * P], t_ps
                    )

            # ---------------- Phase C: kT, qT, V ----------------
            kT = persist.tile([C8, N], F16, tag="kT")
            qT = persist.tile([C8, HALF], F16, tag="qT")
            for seg in range(N // 512):
                kps = psS.tile([P, 1024], F32, tag="s", name=f"k_{seg}")[:C8, :512]
                for cc in range(2):
                    nc.tensor.matmul(
                        kps,
                        wkT[:, cc, :],
                        x2T[:, cc, seg * 512 : (seg + 1) * 512],
                        start=(cc == 0),
                        stop=(cc == 1),
                    )
                nc.vector.tensor_scalar_add(
                    kT[:, seg * 512 : (seg + 1) * 512], kps, bkc[:]
                )
            for seg in range(HALF // 512):
                qps = psS.tile([P, 1024], F32, tag="s", name=f"q_{seg}")[:C8, :512]
                for cc in range(2):
                    nc.tensor.matmul(
                        qps,
                        wqT[:, cc, :],
                        x2T[:, cc, seg * 512 : (seg + 1) * 512],
                        start=(cc == 0),
                        stop=(cc == 1),
                    )
                nc.vector.tensor_scalar_add(
                    qT[:, seg * 512 : (seg + 1) * 512], qps, bqc[:]
                )
            vaug = persist.tile([P, KCH, C + 2], BF16, tag="vaug")
            for nk in range(KCH):
                vps = psS.tile([P, 1024], F32, tag="s", name=f"v_{nk}")[:, : C + 2]
                for cc in range(2):
                    nc.tensor.matmul(
                        vps,
                        x2T[:, cc, nk * P : (nk + 1) * P],
                        wvT[:, cc, :],
                        start=(cc == 0),
                        stop=(cc == 1),
                    )
                nc.vector.tensor_add(vaug[:, nk, :], vps, bvb[:])

            # ---- channel-branch output rows, precomputed into SBUF ----
            # outc_sb[blk] = gamma/2 * attn_c @ x2_rows(blk) + x2_rows(blk)
            # (residual folded in here so the main-loop epilogue is 2 DVE ops)
            outc_sb = persist.tile([P, NBLK, C], F32, tag="outc_sb")
            for blk in range(NBLK):
                c_ps = psS.tile([P, 1024], F32, tag="s", name=f"c_{blk}")[:, :C]
                for dd in range(2):
                    nc.tensor.matmul(
                        c_ps,
                        x2T[:, dd, blk * P : (blk + 1) * P],
                        attnTg[:, dd, :],
                        start=(dd == 0),
                        stop=(dd == 1),
                    )
                nc.vector.tensor_add(
                    outc_sb[:, blk, :], c_ps, x2[:, blk, :]
                )

            # ---------------- Phase D: point attention ----------------
            # 16 rounds per macro block, each covering 2 key chunks with one
            # double-width (2-bank) scores psum + a single wide exp.
            for m in range(HALF // 512):
                o_ps = [
                    psO.tile([P, 512], F32, tag="o", name=f"o_{m}_{j}")[:, : C + 2]
                    for j in range(4)
                ]
                for r in range(KCH // 2):
                    s_ps = psS.tile([P, 1024], F32, tag="s", name=f"s_{m}_{r}")
                    for half in range(2):
                        kk = 2 * r + half
                        nc.tensor.matmul(
                            s_ps[:, half * 512 : (half + 1) * 512],
                            kT[:, kk * P : (kk + 1) * P],
                            qT[:, m * 512 : (m + 1) * 512],
                            start=True,
                            stop=True,
                        )
                    pT = pTp.tile([P, 1024], BF16, tag="pT", name=f"p_{m}_{r}")
                    nc.scalar.activation(pT[:], s_ps[:], ACTF.Exp)
                    for half in range(2):
                        kk = 2 * r + half
                        for j in range(4):
                            nc.tensor.matmul(
                                o_ps[j],
                                pT[:, half * 512 + j * P : half * 512 + (j + 1) * P],
                                vaug[:, kk, :],
                                start=(kk == 0),
                                stop=(kk == KCH - 1),
                            )
                for j in range(4):
                    blk = m * 4 + j
                    # single fast PSUM read frees the O bank for the next
                    # macro block's PV accumulation almost immediately
                    osb = sbout.tile([P, C + 2], F32, tag="osb", name=f"osb{blk}")
                    nc.vector.tensor_copy(osb[:], o_ps[j])
                    rq = small.tile([P, 1], F32, tag="sm", name=f"rq{blk}")
                    nc.vector.reciprocal(rq[:], osb[:, C : C + 1])
                    rqg = small.tile([P, 1], F32, tag="sm", name=f"rqg{blk}")
                    nc.vector.tensor_mul(rqg[:], rq[:], gb[:])
                    acc = sbout.tile([P, C], F32, tag="acc", name=f"acc{blk}")
                    nc.vector.tensor_scalar_mul(acc[:], osb[:, :C], rqg[:])
                    nc.vector.tensor_add(acc[:], acc[:], outc_sb[:, blk, :])
                    nc.sync.dma_start(out_d.ap()[blk], acc[:])

    nc.compile()
    return nc


def _prep_core_inputs(points, Wq, bq, Wk, bk, Wv, bv, gamma, core):
    b, h = core // 2, core % 2
    xb = np.asarray(points[b], dtype=np.float32)
    # own rows first, then the other half (key order is softmax-invariant
    # as long as kT and v use the same order, which they do)
    xp = np.concatenate([xb[h * HALF : (h + 1) * HALF], xb[(1 - h) * HALF : (2 - h) * HALF]])
    x2 = (2.0 * xp).reshape(KCH, P, C).transpose(1, 0, 2).astype(np.float16)  # (128, 32, 256)
    x2T = np.ascontiguousarray(
        (2.0 * xp).T.reshape(2, P, N).transpose(1, 0, 2)
    ).astype(np.float16)  # (128, 2, 4096)
    return {"x2T": x2T, "x2": x2}


def _prep_shared_inputs(Wq, bq, Wk, bk, Wv, bv, gamma):
    wqT = np.ascontiguousarray((0.5 * np.asarray(Wq, np.float32).T).reshape(2, P, C8).transpose(1, 0, 2)).astype(np.float16)
    wkT = np.ascontiguousarray((0.5 * np.asarray(Wk, np.float32).T).reshape(2, P, C8).transpose(1, 0, 2)).astype(np.float16)
    wvT_full = 0.5 * np.asarray(Wv, np.float32).T  # (256, 256)
    wvT_aug = np.concatenate(
        [wvT_full, np.zeros((C, 2), np.float32)], axis=1
    )  # (256, 258)
    wvT = np.ascontiguousarray(wvT_aug.reshape(2, P, C + 2).transpose(1, 0, 2)).astype(np.float16)
    bv_aug = np.concatenate([np.asarray(bv, np.float32), [1.0, 0.0]]).reshape(1, C + 2)
    return {
        "wqT": wqT,
        "wkT": wkT,
        "wvT": wvT,
        "bqc": np.asarray(bq, np.float32).reshape(C8, 1),
        "bkc": np.asarray(bk, np.float32).reshape(C8, 1),
        "bvr": bv_aug,
        "gam": np.asarray(gamma, np.float32).reshape(1, 1),
    }


def kernel(points, Wq, bq, Wk, bk, Wv, bv, gamma, **run_kwargs):
    if "nc" not in _CACHE:
        _CACHE["nc"] = _build_nc()
    nc = _CACHE["nc"]

    shared = _prep_shared_inputs(Wq, bq, Wk, bk, Wv, bv, gamma)
    in_maps = []
    for core in range(NCORES):
        m = dict(shared)
        m.update(_prep_core_inputs(points, Wq, bq, Wk, bk, Wv, bv, gamma, core))
        in_maps.append(m)

    res = run_bass_kernel_spmd(
        nc, in_maps, core_ids=list(range(NCORES)), **run_kwargs
    )
    out = np.empty((B, N, C), dtype=np.float32)
    for core in range(NCORES):
        b, h = core // 2, core % 2
        out[b, h * HALF : (h + 1) * HALF] = (
            res.results[core]["out_rows"].reshape(HALF, C)
        )
    if run_kwargs:
        kernel.last_results = res  # expose profile info to test harness
    return out
